# revision 28
# baseline (speedup 1.0000x reference)
"""Trainium2 Bass kernel for nn_CrossAttention (tanh-scored, reversed-weight
attention), collective-free replicated-KV design, transposed epilogue.

Math (reference):
    q = x1 @ Wq.T + bq ; k = x2 @ Wk.T + bk ; v = x2 @ Wv.T + bv
    attn = softmax(tanh(q @ k.T) / sqrt(512), axis=-1)
    out  = ((1 - attn) / (N-1)) @ v

Kernel algebra:
    t_ij = tanh(q_i . k_j)                        (biases folded into q, k)
    e_ij = exp(scale * t_ij) ~= 1 + scale * t_ij  (|scale*t| <= 0.0442)
    r_i  = N + scale * R_i,  R_i = rowsum_j t_ij
    1/r_i ~= (1/N)(1 - scale*R_i/N)               (|scale*R/N| ~ 2e-6 rel err)
    out_{i,d} = C1_d + c3_d * R_i - k2 * av_{d,i}
      C1_d = cv_d/N + bv_d          (cv = colsum of v-without-bias, fp32 path)
      c3_d = scale*cv_d/(N^2 (N-1))
      k2   = scale/(N (N-1))
      av   = (t @ v)^T  accumulated TRANSPOSED in PSUM ([d partitions, i free])
    The c3_d*R_i correction is folded into the same PSUM accumulation as av
    via one rank-1 f32r matmul per [128,512] output tile (lhsT = c3row
    slice, rhs = R row); a single scalar-engine activation
    (out = -k2*psum + C1_d, per-partition bias) drains PSUM straight to the
    output tile.  v's column 0 is a ones-column, so av^T partition 0 of
    d-chunk 0 IS R_i -- no rowsum matmuls; the dropped true-v column
    contribution is ~2e-8 of out.  C1_col rides a tiny DRAM round-trip to
    reach the d-on-partitions layout the bias needs.

Sharding: rows of x_1 (queries) sharded across 8 cores; x_2/weights
replicated, each core projects full K/V locally; no collectives.

Output is produced transposed per (d-chunk, i-half) tile ([4,2,128,512])
and untransposed host-side in kernel().

DMA queue map (keeps next-iteration loads from stalling behind epilogue):
    sync:   wq, x1, bq loads, then x2 even blocks
    gpsimd: wk, wv, bk, bv, wv32 loads, x2 odd blocks, out DMAs (tail)
    vector: x2 casts cc<4, k drains, rs-row memset/DMA
    scalar: x2 casts cc>=4, v drains, q drains, tanh, cv/coef/epilogue drains
"""

import numpy as np
from contextlib import ExitStack

import concourse.bass as bass
import concourse.mybir as mybir
import concourse.tile as tile
from concourse import bacc
from concourse.bass_utils import run_bass_kernel_spmd

F32 = mybir.dt.float32
BF16 = mybir.dt.bfloat16
FP8 = mybir.dt.float8e4
F32R = mybir.dt.float32r

NCORES = 8
N = 8192             # total rows (keys/values)
CIN = 1024           # input feature dim
D = 512              # d_kq = d_v
P = 128              # partitions
S = N // NCORES      # query rows per core (1024)
NCC = CIN // P       # 8 feature chunks
NDC = D // P         # 4 d chunks
NJB = 16             # x2 streaming blocks
JB = N // NJB        # 512 j columns per block
NJC = N // P         # 64 j chunks
SCALE = float(1.0 / np.sqrt(np.float64(D)))
K2 = float(SCALE / (np.float64(N) * (N - 1)))          # av coefficient
CB = float(-1.0 / N)                                   # c3row coefficient
ACT_COPY = mybir.ActivationFunctionType.Copy
ACT_IDENT = mybir.ActivationFunctionType.Identity
ACT_TANH = mybir.ActivationFunctionType.Tanh
DR = mybir.MatmulPerfMode.DoubleRow


def emit_body(nc, tc, io, persist_tiles, pools, staged=False):
    """Emit one full kernel iteration (projections + attention + epilogue).

    staged=True places the three staggered-reset stage boundaries right
    after the load/projection phases (none inside the attention loop), so
    the next iteration's stage-0 (weights, x1 and the first four x2 block
    DMAs) fires once this iteration's kv-stream (stage 2) retires --
    during the main attention loop -- instead of near the iteration end.
    """
    x1t, x2t, wqt, wkt, wvt, wvt32, bqt, bkt, bvt, out = io
    (wpool, kvpool, tfull, loads8, loadsbf, epool, cspool, ps2,
     dpool) = pools

    # ---- weight / bias / x1 loads; q-side on sync (fires earliest) ----
    wq_sb = wpool.tile([P, NCC, D], FP8, tag="wq")
    wk_sb = wpool.tile([P, NCC, D], FP8, tag="wk")
    wv_sb = wpool.tile([P, NCC, D], FP8, tag="wv")
    wv32_sb = wpool.tile([P, NCC, D], F32, tag="wv32")
    bq_sb = wpool.tile([P, NDC], F32, tag="bq")
    bk_sb = wpool.tile([P, NDC], F32, tag="bk")
    bvt_sb = wpool.tile([P, NDC], F32, tag="bvt")
    x1_sb = wpool.tile([P, NCC, S], FP8, tag="x1")
    nc.sync.dma_start(out=wq_sb, in_=wqt[:, :, :])
    nc.sync.dma_start(out=x1_sb, in_=x1t[:, :, :])
    nc.sync.dma_start(out=bq_sb, in_=bqt[:, :])
    nc.gpsimd.dma_start(out=wk_sb, in_=wkt[:, :, :])
    nc.gpsimd.dma_start(out=wv_sb, in_=wvt[:, :, :])
    nc.gpsimd.dma_start(out=bk_sb, in_=bkt[:, :])
    nc.gpsimd.dma_start(out=bvt_sb, in_=bvt[:, :])

    # first 4 x2 blocks load in stage 0 alongside the weights
    x2b_pre = []
    for jb in range(4):
        x2b = loadsbf.tile([P, NCC, JB], BF16, tag="x2b", name=f"x2b_p{jb}")
        if jb % 2 == 0:
            nc.sync.dma_start(out=x2b, in_=x2t[jb, :, :, :])
        else:
            nc.gpsimd.dma_start(out=x2b, in_=x2t[jb, :, :, :])
        x2b_pre.append(x2b)

    # ---- q projection: qt[d, i] fp8, bias folded ----
    qt = kvpool.tile([P, NDC, S], FP8, tag="qt")
    for di in range(NDC):
        pq = ps2.tile([P, 2, D], F32, tag="s2")
        for ih in range(2):
            for cp in range(NCC // 2):
                nc.tensor.matmul(
                    pq[:, ih, :],
                    lhsT=wq_sb[:, 2 * cp:2 * cp + 2, di * P:(di + 1) * P],
                    rhs=x1_sb[:, 2 * cp:2 * cp + 2, ih * D:(ih + 1) * D],
                    perf_mode=DR, start=(cp == 0), stop=(cp == NCC // 2 - 1))
        nc.scalar.activation(out=qt[:, di, :], in_=pq,
                             func=ACT_IDENT, bias=bq_sb[:, di:di + 1])

    if staged:
        tc.stage_boundary()

    # ---- streamed k/v projection over 16 j-blocks of 512 ----
    kt = kvpool.tile([P, NDC, N], FP8, tag="kt")        # kT[d, j]
    vv = kvpool.tile([P, NJC, D], FP8, tag="v")         # v[j, d] (no bias)
    # v column 0 is a ones-column: (t@v)^T then lands R_i (the rowsum of t)
    # at av^T partition 0 of d-chunk 0, where a plain (no partition shift)
    # activation copy can read it into the epilogue's rhs row 0.
    nc.vector.memset(vv[:, :, 0:1], 1.0)
    cs_part = cspool.tile([P, NCC, NJB], F32, tag="csp")
    for jb in range(NJB):
        if staged and jb == NJB // 2:
            tc.stage_boundary()
        if jb < 4:
            x2b = x2b_pre[jb]
        else:
            x2b = loadsbf.tile([P, NCC, JB], BF16, tag="x2b")
            # alternate queues so the 16 MB stream rides two DMA channels
            if jb % 2 == 0:
                nc.sync.dma_start(out=x2b, in_=x2t[jb, :, :, :])
            else:
                nc.gpsimd.dma_start(out=x2b, in_=x2t[jb, :, :, :])
        x2f = loads8.tile([P, NCC, JB], FP8, tag="x2f")
        # fused bf16->fp8 cast + f32-accumulated colsum via accum_out
        for cc in range(NCC):
            if cc < 4:
                nc.vector.tensor_scalar(
                    x2f[:, cc, :], x2b[:, cc, :], 1.0, 0.0,
                    op0=mybir.AluOpType.mult, op1=mybir.AluOpType.add,
                    accum_out=cs_part[:, cc, jb:jb + 1])
            else:
                nc.scalar.activation(
                    out=x2f[:, cc, :], in_=x2b[:, cc, :], func=ACT_COPY,
                    accum_out=cs_part[:, cc, jb:jb + 1])
        # kT block: [512 d, 512 j]
        for dp in range(NDC // 2):
            pk = ps2.tile([P, 2, D], F32, tag="s2")
            for dh in range(2):
                di = 2 * dp + dh
                for cp in range(NCC // 2):
                    nc.tensor.matmul(
                        pk[:, dh, :],
                        lhsT=wk_sb[:, 2 * cp:2 * cp + 2, di * P:(di + 1) * P],
                        rhs=x2f[:, 2 * cp:2 * cp + 2, :],
                        perf_mode=DR, start=(cp == 0),
                        stop=(cp == NCC // 2 - 1))
                nc.vector.tensor_scalar_add(
                    kt[:, di, jb * JB:(jb + 1) * JB], pk[:, dh, :],
                    bk_sb[:, di:di + 1])
        # v block: [512 j, 512 dv]
        for vp in range(JB // P // 2):
            pv = ps2.tile([P, 2, D], F32, tag="s2")
            for vh in range(2):
                jl = 2 * vp + vh
                for cp in range(NCC // 2):
                    nc.tensor.matmul(
                        pv[:, vh, :],
                        lhsT=x2f[:, 2 * cp:2 * cp + 2, jl * P:(jl + 1) * P],
                        rhs=wv_sb[:, 2 * cp:2 * cp + 2, :],
                        perf_mode=DR, start=(cp == 0),
                        stop=(cp == NCC // 2 - 1))
            jj = jb * (JB // P) + 2 * vp
            nc.scalar.activation(out=vv[:, jj:jj + 2, 1:D],
                                 in_=pv[:, :, 1:D], func=ACT_COPY)

    # wv32 is only needed now (cv); its load rides after the stream dispatches
    nc.gpsimd.dma_start(out=wv32_sb, in_=wvt32[:, :, :])

    # ---- colsum_v (fp32): cv1[1,512], then coef[2,512] via tiny PE matmuls
    cs = cspool.tile([P, NCC], F32, tag="cs")
    nc.vector.reduce_sum(out=cs, in_=cs_part, axis=mybir.AxisListType.X)
    ps_cv = ps2.tile([P, 2, D], F32, tag="s2")
    for ci in range(NCC):
        nc.tensor.matmul(ps_cv[0:1, 0, :], lhsT=cs[:, ci:ci + 1],
                         rhs=wv32_sb[:, ci, :],
                         start=(ci == 0), stop=(ci == NCC - 1))
    cv1 = cspool.tile([1, D], F32, tag="cv1")
    nc.scalar.activation(out=cv1, in_=ps_cv[0:1, 0, :], func=ACT_COPY)
    # c3row = -cv/N (rank-1 coefficient row, partition 0)
    c3row = cspool.tile([1, D], F32R, tag="c3row")
    nc.vector.tensor_scalar_mul(c3row, cv1, CB)
    # C1_col[128, NDC] = cv^T/N + bv^T; the [1,512]->[128,4] transpose rides
    # a tiny DRAM round-trip (DMA reshapes across partitions freely)
    cv_dram = dpool.tile([1, D], F32, name="cvd")
    nc.scalar.dma_start(out=cv_dram, in_=cv1)
    cvt_col = cspool.tile([P, NDC], F32, tag="cvtc")
    for dc in range(NDC):
        nc.scalar.dma_start(out=cvt_col[:, dc:dc + 1],
                            in_=cv_dram[0:1, dc * P:(dc + 1) * P])
    c1col = cspool.tile([P, NDC], F32, tag="c1col")
    nc.vector.tensor_scalar_mul(c1col, cvt_col, float(1.0 / N))
    nc.vector.tensor_add(c1col, c1col, bvt_sb)

    if staged:
        tc.stage_boundary()

    # ---- main attention loop, one i-half (512 rows) at a time ----
    for ih in range(2):
        ps_av = [ps2.tile([P, 2, D], F32, tag="s2", name=f"av{ap}_{ih}")
                 for ap in range(2)]
        t2f = tfull.tile([P, NJC, D], FP8, tag="t2f")
        for jp in range(NJC // 2):
            ps_s = ps2.tile([P, 2, D], F32, tag="s2")
            for sh in range(2):
                jc = 2 * jp + sh
                for qp in range(2):
                    nc.tensor.matmul(
                        ps_s[:, sh, :],
                        lhsT=kt[:, 2 * qp:2 * qp + 2, jc * P:(jc + 1) * P],
                        rhs=qt[:, 2 * qp:2 * qp + 2, ih * D:(ih + 1) * D],
                        perf_mode=DR, start=(qp == 0), stop=(qp == 1))
            nc.scalar.activation(out=t2f[:, 2 * jp:2 * jp + 2, :], in_=ps_s,
                                 func=ACT_TANH)
            # av^T accumulation: [d partitions, i free]
            for dc in range(NDC):
                nc.tensor.matmul(
                    ps_av[dc // 2][:, dc % 2, :],
                    lhsT=vv[:, 2 * jp:2 * jp + 2, dc * P:(dc + 1) * P],
                    rhs=t2f[:, 2 * jp:2 * jp + 2, :],
                    perf_mode=DR, start=(jp == 0), stop=False)

        # ---- epilogue: R_i sits at av^T partition 0 of d-chunk 0 ----
        rs_row = epool.tile([1, D], F32R, tag="rs")
        nc.scalar.activation(out=rs_row, in_=ps_av[0][0:1, 0, :],
                             func=ACT_COPY)
        for dc in range(NDC):
            pa = ps_av[dc // 2][:, dc % 2, :]
            # fold the rank-1 R-term into the accumulation, then drain with
            # out = -K2 * psum + C1_d  (per-partition bias) on the scalar eng
            nc.tensor.matmul(pa, lhsT=c3row[0:1, dc * P:(dc + 1) * P],
                             rhs=rs_row, start=False, stop=True)
            o1 = epool.tile([P, D], F32, tag="o1")
            # drains split across engines so all four PSUM banks free fast
            # (the next iteration's q-projection reuses them)
            if dc < 2:
                nc.vector.tensor_scalar(
                    o1, pa, -K2, c1col[:, dc:dc + 1],
                    op0=mybir.AluOpType.mult, op1=mybir.AluOpType.add)
            else:
                nc.scalar.activation(out=o1, in_=pa, func=ACT_IDENT,
                                     scale=-K2, bias=c1col[:, dc:dc + 1])
            nc.gpsimd.dma_start(out=out[dc, ih, :, :], in_=o1)


def build_kernel(repeat: int = 1):
    nc = bacc.Bacc(num_devices=NCORES)

    x1t = nc.declare_dram_parameter("x1t", [P, NCC, S], FP8, isOutput=False)
    x2t = nc.declare_dram_parameter("x2t", [NJB, P, NCC, JB], BF16,
                                    isOutput=False)
    wqt = nc.declare_dram_parameter("wqt", [P, NCC, D], FP8, isOutput=False)
    wkt = nc.declare_dram_parameter("wkt", [P, NCC, D], FP8, isOutput=False)
    wvt = nc.declare_dram_parameter("wvt", [P, NCC, D], FP8, isOutput=False)
    wvt32 = nc.declare_dram_parameter("wvt32", [P, NCC, D], F32, isOutput=False)
    bqt = nc.declare_dram_parameter("bqt", [P, NDC], F32, isOutput=False)
    bkt = nc.declare_dram_parameter("bkt", [P, NDC], F32, isOutput=False)
    bvt = nc.declare_dram_parameter("bvt", [P, NDC], F32, isOutput=False)
    # transposed output tiles: [d-chunk, i-half, 128 d, 512 i]
    out = nc.declare_dram_parameter("out", [NDC, 2, P, D], F32, isOutput=True)
    io = (x1t, x2t, wqt, wkt, wvt, wvt32, bqt, bkt, bvt, out)

    with tile.TileContext(nc) as tc, ExitStack() as ctx:
        persist_tiles = ()

        wpool = ctx.enter_context(tc.tile_pool(name="weights", bufs=1))
        kvpool = ctx.enter_context(tc.tile_pool(name="kv", bufs=1))
        tfull = ctx.enter_context(tc.tile_pool(name="tfull", bufs=1))
        loads8 = ctx.enter_context(tc.tile_pool(name="loads8", bufs=4))
        loadsbf = ctx.enter_context(tc.tile_pool(name="loadsbf", bufs=4))
        epool = ctx.enter_context(tc.tile_pool(name="epool", bufs=2))
        cspool = ctx.enter_context(tc.tile_pool(name="cspool", bufs=1))
        ps2 = ctx.enter_context(tc.tile_pool(name="ps2", bufs=4, space="PSUM"))
        dpool = ctx.enter_context(tc.tile_pool(name="dram", bufs=2,
                                               space="DRAM"))
        pools = (wpool, kvpool, tfull, loads8, loadsbf, epool, cspool, ps2,
                 dpool)

        if repeat == 1:
            emit_body(nc, tc, io, persist_tiles, pools)
        else:
            with tc.For_i(0, repeat, 1,
                          staggered_reset=True,
                          hint_engines=(mybir.EngineType.PE,
                                        mybir.EngineType.Activation,
                                        mybir.EngineType.DVE,
                                        mybir.EngineType.SP,
                                        mybir.EngineType.Pool)):
                emit_body(nc, tc, io, persist_tiles, pools, staged=True)

    if not nc.is_finalized():
        nc.finalize()
    return nc


_NC_CACHE = {}


def _get_nc(repeat: int = 1):
    if repeat not in _NC_CACHE:
        _NC_CACHE[repeat] = build_kernel(repeat)
    return _NC_CACHE[repeat]


def make_in_maps(x_1, x_2, Wq, bq, Wk, bk, Wv, bv):
    f8 = mybir.dt.np(FP8)
    bf = mybir.dt.np(BF16)

    def chunked_t(a, dtype):
        # [rows, cin] -> transposed, feature-chunked [128, cin//128, rows]
        a = np.asarray(a, np.float32)
        cin, rows = a.shape[1], a.shape[0]
        return np.ascontiguousarray(
            a.T.reshape(cin // P, P, rows).transpose(1, 0, 2)).astype(dtype)

    def blocked(a):
        # [128, 8, N] -> j-blocked [16, 128, 8, 512] (contiguous per block)
        return np.ascontiguousarray(
            a.reshape(P, NCC, NJB, JB).transpose(2, 0, 1, 3))

    x1t = chunked_t(x_1, f8)                      # [128, 8, 8192]
    shared = {
        "x2t": blocked(chunked_t(x_2, bf)),
        "wqt": chunked_t(np.asarray(Wq), f8),     # [128, 8, 512]
        "wkt": chunked_t(np.asarray(Wk), f8),
        "wvt": chunked_t(np.asarray(Wv), f8),
        "wvt32": chunked_t(np.asarray(Wv), np.float32),
        "bqt": np.ascontiguousarray(
            np.asarray(bq, np.float32).reshape(NDC, P).T),
        "bkt": np.ascontiguousarray(
            np.asarray(bk, np.float32).reshape(NDC, P).T),
        "bvt": np.ascontiguousarray(
            np.asarray(bv, np.float32).reshape(NDC, P).T),
    }
    return [
        {"x1t": np.ascontiguousarray(x1t[:, :, c * S:(c + 1) * S]), **shared}
        for c in range(NCORES)
    ]


def untranspose_out(o):
    # [NDC, 2, 128 d, 512 i] -> [1024 i, 512 d]
    return np.ascontiguousarray(
        o.transpose(1, 3, 0, 2).reshape(S, D))


def kernel(x_1, x_2, Wq, bq, Wk, bk, Wv, bv):
    nc = _get_nc(1)
    in_maps = make_in_maps(x_1, x_2, Wq, bq, Wk, bk, Wv, bv)
    res = run_bass_kernel_spmd(nc, in_maps, core_ids=list(range(NCORES)))
    return np.concatenate(
        [untranspose_out(res.results[c]["out"]) for c in range(NCORES)],
        axis=0)


# revision 29
# speedup vs baseline: 1.0493x; 1.0493x over previous
"""Trainium2 Bass kernel for nn_CrossAttention (tanh-scored, reversed-weight
attention), collective-free replicated-KV design, transposed epilogue.

Math (reference):
    q = x1 @ Wq.T + bq ; k = x2 @ Wk.T + bk ; v = x2 @ Wv.T + bv
    attn = softmax(tanh(q @ k.T) / sqrt(512), axis=-1)
    out  = ((1 - attn) / (N-1)) @ v

Kernel algebra:
    t_ij = tanh(q_i . k_j)                        (biases folded into q, k)
    e_ij = exp(scale * t_ij) ~= 1 + scale * t_ij  (|scale*t| <= 0.0442)
    r_i  = N + scale * R_i,  R_i = rowsum_j t_ij
    1/r_i ~= (1/N)(1 - scale*R_i/N)               (|scale*R/N| ~ 2e-6 rel err)
    out_{i,d} = C1_d + c3_d * R_i - k2 * av_{d,i}
      C1_d = cv_d/N + bv_d          (cv = colsum of v-without-bias, fp32 path)
      c3_d = scale*cv_d/(N^2 (N-1))
      k2   = scale/(N (N-1))
      av   = (t @ v)^T  accumulated TRANSPOSED in PSUM ([d partitions, i free])
    The c3_d*R_i correction is folded into the same PSUM accumulation as av
    via one rank-1 f32r matmul per [128,512] output tile (lhsT = c3row
    slice, rhs = R row); a single scalar-engine activation
    (out = -k2*psum + C1_d, per-partition bias) drains PSUM straight to the
    output tile.  v's column 0 is a ones-column, so av^T partition 0 of
    d-chunk 0 IS R_i -- no rowsum matmuls; the dropped true-v column
    contribution is ~2e-8 of out.  C1_col rides a tiny DRAM round-trip to
    reach the d-on-partitions layout the bias needs.

Sharding: rows of x_1 (queries) sharded across 8 cores; x_2/weights
replicated, each core projects full K/V locally; no collectives.

Output is produced transposed per (d-chunk, i-half) tile ([4,2,128,512])
and untransposed host-side in kernel().

DMA queue map (keeps next-iteration loads from stalling behind epilogue):
    sync:   wq, x1, bq loads, then x2 even blocks
    gpsimd: wk, wv, bk, bv, wv32 loads, x2 odd blocks, out DMAs (tail)
    vector: x2 casts cc<4, k drains, rs-row memset/DMA
    scalar: x2 casts cc>=4, v drains, q drains, tanh, cv/coef/epilogue drains
"""

import numpy as np
from contextlib import ExitStack

import concourse.bass as bass
import concourse.mybir as mybir
import concourse.tile as tile
from concourse import bacc
from concourse.bass_utils import run_bass_kernel_spmd

F32 = mybir.dt.float32
BF16 = mybir.dt.bfloat16
FP8 = mybir.dt.float8e4
F32R = mybir.dt.float32r

NCORES = 8
N = 8192             # total rows (keys/values)
CIN = 1024           # input feature dim
D = 512              # d_kq = d_v
P = 128              # partitions
S = N // NCORES      # query rows per core (1024)
NCC = CIN // P       # 8 feature chunks
NDC = D // P         # 4 d chunks
NJB = 16             # x2 streaming blocks
JB = N // NJB        # 512 j columns per block
NJC = N // P         # 64 j chunks
SCALE = float(1.0 / np.sqrt(np.float64(D)))
K2 = float(SCALE / (np.float64(N) * (N - 1)))          # av coefficient
CB = float(-1.0 / N)                                   # c3row coefficient
ACT_COPY = mybir.ActivationFunctionType.Copy
ACT_IDENT = mybir.ActivationFunctionType.Identity
ACT_TANH = mybir.ActivationFunctionType.Tanh
DR = mybir.MatmulPerfMode.DoubleRow


def emit_body(nc, tc, io, persist_tiles, pools, staged=False):
    """Emit one full kernel iteration (projections + attention + epilogue).

    The first four x2 block DMAs are hoisted ahead of the q-projection so
    they sit early in the trigger queues and the staggered-reset stage
    assignment, letting the next iteration's stream prefetch sooner.
    (Manual tc.stage_boundary() placements were tried twice and always
    regressed ~15us -- the per-stage barrier overhead exceeds the gain.)
    """
    x1t, x2t, wqt, wkt, wvt, wvt32, bqt, bkt, bvt, out = io
    (wpool, kvpool, tfull, loads8, loadsbf, epool, cspool, ps2,
     dpool) = pools

    # ---- weight / bias / x1 loads; q-side on sync (fires earliest) ----
    wq_sb = wpool.tile([P, NCC, D], FP8, tag="wq")
    wk_sb = wpool.tile([P, NCC, D], FP8, tag="wk")
    wv_sb = wpool.tile([P, NCC, D], FP8, tag="wv")
    wv32_sb = wpool.tile([P, NCC, D], F32, tag="wv32")
    bq_sb = wpool.tile([P, NDC], F32, tag="bq")
    bk_sb = wpool.tile([P, NDC], F32, tag="bk")
    bvt_sb = wpool.tile([P, NDC], F32, tag="bvt")
    x1_sb = wpool.tile([P, NCC, S], FP8, tag="x1")
    nc.sync.dma_start(out=wq_sb, in_=wqt[:, :, :])
    nc.sync.dma_start(out=x1_sb, in_=x1t[:, :, :])
    nc.sync.dma_start(out=bq_sb, in_=bqt[:, :])
    nc.gpsimd.dma_start(out=wk_sb, in_=wkt[:, :, :])
    nc.gpsimd.dma_start(out=wv_sb, in_=wvt[:, :, :])
    nc.gpsimd.dma_start(out=bk_sb, in_=bkt[:, :])
    nc.gpsimd.dma_start(out=bvt_sb, in_=bvt[:, :])

    # first 4 x2 blocks load in stage 0 alongside the weights
    x2b_pre = []
    for jb in range(4):
        x2b = loadsbf.tile([P, NCC, JB], BF16, tag="x2b", name=f"x2b_p{jb}")
        if jb % 2 == 0:
            nc.sync.dma_start(out=x2b, in_=x2t[jb, :, :, :])
        else:
            nc.gpsimd.dma_start(out=x2b, in_=x2t[jb, :, :, :])
        x2b_pre.append(x2b)

    # ---- q projection: qt[d, i] fp8, bias folded ----
    qt = kvpool.tile([P, NDC, S], FP8, tag="qt")
    for di in range(NDC):
        pq = ps2.tile([P, 2, D], F32, tag="s2")
        for ih in range(2):
            for cp in range(NCC // 2):
                nc.tensor.matmul(
                    pq[:, ih, :],
                    lhsT=wq_sb[:, 2 * cp:2 * cp + 2, di * P:(di + 1) * P],
                    rhs=x1_sb[:, 2 * cp:2 * cp + 2, ih * D:(ih + 1) * D],
                    perf_mode=DR, start=(cp == 0), stop=(cp == NCC // 2 - 1))
        nc.scalar.activation(out=qt[:, di, :], in_=pq,
                             func=ACT_IDENT, bias=bq_sb[:, di:di + 1])

    # ---- streamed k/v projection over 16 j-blocks of 512 ----
    kt = kvpool.tile([P, NDC, N], FP8, tag="kt")        # kT[d, j]
    vv = kvpool.tile([P, NJC, D], FP8, tag="v")         # v[j, d] (no bias)
    # v column 0 is a ones-column: (t@v)^T then lands R_i (the rowsum of t)
    # at av^T partition 0 of d-chunk 0, where a plain (no partition shift)
    # activation copy can read it into the epilogue's rhs row 0.
    nc.vector.memset(vv[:, :, 0:1], 1.0)
    cs_part = cspool.tile([P, NCC, NJB], F32, tag="csp")
    for jb in range(NJB):
        if jb < 4:
            x2b = x2b_pre[jb]
        else:
            x2b = loadsbf.tile([P, NCC, JB], BF16, tag="x2b")
            # alternate queues so the 16 MB stream rides two DMA channels
            if jb % 2 == 0:
                nc.sync.dma_start(out=x2b, in_=x2t[jb, :, :, :])
            else:
                nc.gpsimd.dma_start(out=x2b, in_=x2t[jb, :, :, :])
        x2f = loads8.tile([P, NCC, JB], FP8, tag="x2f")
        # fused bf16->fp8 cast + f32-accumulated colsum via accum_out
        for cc in range(NCC):
            if cc < 4:
                nc.vector.tensor_scalar(
                    x2f[:, cc, :], x2b[:, cc, :], 1.0, 0.0,
                    op0=mybir.AluOpType.mult, op1=mybir.AluOpType.add,
                    accum_out=cs_part[:, cc, jb:jb + 1])
            else:
                nc.scalar.activation(
                    out=x2f[:, cc, :], in_=x2b[:, cc, :], func=ACT_COPY,
                    accum_out=cs_part[:, cc, jb:jb + 1])
        # kT block: [512 d, 512 j]
        for dp in range(NDC // 2):
            pk = ps2.tile([P, 2, D], F32, tag="s2")
            for dh in range(2):
                di = 2 * dp + dh
                for cp in range(NCC // 2):
                    nc.tensor.matmul(
                        pk[:, dh, :],
                        lhsT=wk_sb[:, 2 * cp:2 * cp + 2, di * P:(di + 1) * P],
                        rhs=x2f[:, 2 * cp:2 * cp + 2, :],
                        perf_mode=DR, start=(cp == 0),
                        stop=(cp == NCC // 2 - 1))
                nc.vector.tensor_scalar_add(
                    kt[:, di, jb * JB:(jb + 1) * JB], pk[:, dh, :],
                    bk_sb[:, di:di + 1])
        # v block: [512 j, 512 dv]
        for vp in range(JB // P // 2):
            pv = ps2.tile([P, 2, D], F32, tag="s2")
            for vh in range(2):
                jl = 2 * vp + vh
                for cp in range(NCC // 2):
                    nc.tensor.matmul(
                        pv[:, vh, :],
                        lhsT=x2f[:, 2 * cp:2 * cp + 2, jl * P:(jl + 1) * P],
                        rhs=wv_sb[:, 2 * cp:2 * cp + 2, :],
                        perf_mode=DR, start=(cp == 0),
                        stop=(cp == NCC // 2 - 1))
            jj = jb * (JB // P) + 2 * vp
            nc.scalar.activation(out=vv[:, jj:jj + 2, 1:D],
                                 in_=pv[:, :, 1:D], func=ACT_COPY)

    # wv32 is only needed now (cv); its load rides after the stream dispatches
    nc.gpsimd.dma_start(out=wv32_sb, in_=wvt32[:, :, :])

    # ---- colsum_v (fp32): cv1[1,512], then coef[2,512] via tiny PE matmuls
    cs = cspool.tile([P, NCC], F32, tag="cs")
    nc.vector.reduce_sum(out=cs, in_=cs_part, axis=mybir.AxisListType.X)
    ps_cv = ps2.tile([P, 2, D], F32, tag="s2")
    for ci in range(NCC):
        nc.tensor.matmul(ps_cv[0:1, 0, :], lhsT=cs[:, ci:ci + 1],
                         rhs=wv32_sb[:, ci, :],
                         start=(ci == 0), stop=(ci == NCC - 1))
    cv1 = cspool.tile([1, D], F32, tag="cv1")
    nc.scalar.activation(out=cv1, in_=ps_cv[0:1, 0, :], func=ACT_COPY)
    # c3row = -cv/N (rank-1 coefficient row, partition 0)
    c3row = cspool.tile([1, D], F32R, tag="c3row")
    nc.vector.tensor_scalar_mul(c3row, cv1, CB)
    # C1_col[128, NDC] = cv^T/N + bv^T; the [1,512]->[128,4] transpose rides
    # a tiny DRAM round-trip (DMA reshapes across partitions freely)
    cv_dram = dpool.tile([1, D], F32, name="cvd")
    nc.scalar.dma_start(out=cv_dram, in_=cv1)
    cvt_col = cspool.tile([P, NDC], F32, tag="cvtc")
    for dc in range(NDC):
        nc.scalar.dma_start(out=cvt_col[:, dc:dc + 1],
                            in_=cv_dram[0:1, dc * P:(dc + 1) * P])
    c1col = cspool.tile([P, NDC], F32, tag="c1col")
    nc.vector.tensor_scalar_mul(c1col, cvt_col, float(1.0 / N))
    nc.vector.tensor_add(c1col, c1col, bvt_sb)

    # ---- main attention loop, one i-half (512 rows) at a time ----
    for ih in range(2):
        ps_av = [ps2.tile([P, 2, D], F32, tag="s2", name=f"av{ap}_{ih}")
                 for ap in range(2)]
        t2f = tfull.tile([P, NJC, D], FP8, tag="t2f")
        for jp in range(NJC // 2):
            ps_s = ps2.tile([P, 2, D], F32, tag="s2")
            for sh in range(2):
                jc = 2 * jp + sh
                for qp in range(2):
                    nc.tensor.matmul(
                        ps_s[:, sh, :],
                        lhsT=kt[:, 2 * qp:2 * qp + 2, jc * P:(jc + 1) * P],
                        rhs=qt[:, 2 * qp:2 * qp + 2, ih * D:(ih + 1) * D],
                        perf_mode=DR, start=(qp == 0), stop=(qp == 1))
            nc.scalar.activation(out=t2f[:, 2 * jp:2 * jp + 2, :], in_=ps_s,
                                 func=ACT_TANH)
            # av^T accumulation: [d partitions, i free]
            for dc in range(NDC):
                nc.tensor.matmul(
                    ps_av[dc // 2][:, dc % 2, :],
                    lhsT=vv[:, 2 * jp:2 * jp + 2, dc * P:(dc + 1) * P],
                    rhs=t2f[:, 2 * jp:2 * jp + 2, :],
                    perf_mode=DR, start=(jp == 0), stop=False)

        # ---- epilogue: R_i sits at av^T partition 0 of d-chunk 0 ----
        rs_row = epool.tile([1, D], F32R, tag="rs")
        nc.scalar.activation(out=rs_row, in_=ps_av[0][0:1, 0, :],
                             func=ACT_COPY)
        for dc in range(NDC):
            pa = ps_av[dc // 2][:, dc % 2, :]
            # fold the rank-1 R-term into the accumulation, then drain with
            # out = -K2 * psum + C1_d  (per-partition bias) on the scalar eng
            nc.tensor.matmul(pa, lhsT=c3row[0:1, dc * P:(dc + 1) * P],
                             rhs=rs_row, start=False, stop=True)
            o1 = epool.tile([P, D], F32, tag="o1")
            # drains split across engines so all four PSUM banks free fast
            # (the next iteration's q-projection reuses them)
            if dc < 2:
                nc.vector.tensor_scalar(
                    o1, pa, -K2, c1col[:, dc:dc + 1],
                    op0=mybir.AluOpType.mult, op1=mybir.AluOpType.add)
            else:
                nc.scalar.activation(out=o1, in_=pa, func=ACT_IDENT,
                                     scale=-K2, bias=c1col[:, dc:dc + 1])
            nc.gpsimd.dma_start(out=out[dc, ih, :, :], in_=o1)


def build_kernel(repeat: int = 1):
    nc = bacc.Bacc(num_devices=NCORES)

    x1t = nc.declare_dram_parameter("x1t", [P, NCC, S], FP8, isOutput=False)
    x2t = nc.declare_dram_parameter("x2t", [NJB, P, NCC, JB], BF16,
                                    isOutput=False)
    wqt = nc.declare_dram_parameter("wqt", [P, NCC, D], FP8, isOutput=False)
    wkt = nc.declare_dram_parameter("wkt", [P, NCC, D], FP8, isOutput=False)
    wvt = nc.declare_dram_parameter("wvt", [P, NCC, D], FP8, isOutput=False)
    wvt32 = nc.declare_dram_parameter("wvt32", [P, NCC, D], F32, isOutput=False)
    bqt = nc.declare_dram_parameter("bqt", [P, NDC], F32, isOutput=False)
    bkt = nc.declare_dram_parameter("bkt", [P, NDC], F32, isOutput=False)
    bvt = nc.declare_dram_parameter("bvt", [P, NDC], F32, isOutput=False)
    # transposed output tiles: [d-chunk, i-half, 128 d, 512 i]
    out = nc.declare_dram_parameter("out", [NDC, 2, P, D], F32, isOutput=True)
    io = (x1t, x2t, wqt, wkt, wvt, wvt32, bqt, bkt, bvt, out)

    with tile.TileContext(nc) as tc, ExitStack() as ctx:
        persist_tiles = ()

        wpool = ctx.enter_context(tc.tile_pool(name="weights", bufs=1))
        kvpool = ctx.enter_context(tc.tile_pool(name="kv", bufs=1))
        tfull = ctx.enter_context(tc.tile_pool(name="tfull", bufs=1))
        loads8 = ctx.enter_context(tc.tile_pool(name="loads8", bufs=4))
        loadsbf = ctx.enter_context(tc.tile_pool(name="loadsbf", bufs=4))
        epool = ctx.enter_context(tc.tile_pool(name="epool", bufs=2))
        cspool = ctx.enter_context(tc.tile_pool(name="cspool", bufs=1))
        ps2 = ctx.enter_context(tc.tile_pool(name="ps2", bufs=4, space="PSUM"))
        dpool = ctx.enter_context(tc.tile_pool(name="dram", bufs=2,
                                               space="DRAM"))
        pools = (wpool, kvpool, tfull, loads8, loadsbf, epool, cspool, ps2,
                 dpool)

        if repeat == 1:
            emit_body(nc, tc, io, persist_tiles, pools)
        else:
            with tc.For_i(0, repeat, 1,
                          staggered_reset=True,
                          hint_engines=(mybir.EngineType.PE,
                                        mybir.EngineType.Activation,
                                        mybir.EngineType.DVE,
                                        mybir.EngineType.SP,
                                        mybir.EngineType.Pool)):
                emit_body(nc, tc, io, persist_tiles, pools, staged=True)

    if not nc.is_finalized():
        nc.finalize()
    return nc


_NC_CACHE = {}


def _get_nc(repeat: int = 1):
    if repeat not in _NC_CACHE:
        _NC_CACHE[repeat] = build_kernel(repeat)
    return _NC_CACHE[repeat]


def make_in_maps(x_1, x_2, Wq, bq, Wk, bk, Wv, bv):
    f8 = mybir.dt.np(FP8)
    bf = mybir.dt.np(BF16)

    def chunked_t(a, dtype):
        # [rows, cin] -> transposed, feature-chunked [128, cin//128, rows]
        a = np.asarray(a, np.float32)
        cin, rows = a.shape[1], a.shape[0]
        return np.ascontiguousarray(
            a.T.reshape(cin // P, P, rows).transpose(1, 0, 2)).astype(dtype)

    def blocked(a):
        # [128, 8, N] -> j-blocked [16, 128, 8, 512] (contiguous per block)
        return np.ascontiguousarray(
            a.reshape(P, NCC, NJB, JB).transpose(2, 0, 1, 3))

    x1t = chunked_t(x_1, f8)                      # [128, 8, 8192]
    shared = {
        "x2t": blocked(chunked_t(x_2, bf)),
        "wqt": chunked_t(np.asarray(Wq), f8),     # [128, 8, 512]
        "wkt": chunked_t(np.asarray(Wk), f8),
        "wvt": chunked_t(np.asarray(Wv), f8),
        "wvt32": chunked_t(np.asarray(Wv), np.float32),
        "bqt": np.ascontiguousarray(
            np.asarray(bq, np.float32).reshape(NDC, P).T),
        "bkt": np.ascontiguousarray(
            np.asarray(bk, np.float32).reshape(NDC, P).T),
        "bvt": np.ascontiguousarray(
            np.asarray(bv, np.float32).reshape(NDC, P).T),
    }
    return [
        {"x1t": np.ascontiguousarray(x1t[:, :, c * S:(c + 1) * S]), **shared}
        for c in range(NCORES)
    ]


def untranspose_out(o):
    # [NDC, 2, 128 d, 512 i] -> [1024 i, 512 d]
    return np.ascontiguousarray(
        o.transpose(1, 3, 0, 2).reshape(S, D))


def kernel(x_1, x_2, Wq, bq, Wk, bk, Wv, bv):
    nc = _get_nc(1)
    in_maps = make_in_maps(x_1, x_2, Wq, bq, Wk, bk, Wv, bv)
    res = run_bass_kernel_spmd(nc, in_maps, core_ids=list(range(NCORES)))
    return np.concatenate(
        [untranspose_out(res.results[c]["out"]) for c in range(NCORES)],
        axis=0)


# revision 30
# speedup vs baseline: 1.0744x; 1.0239x over previous
"""Trainium2 Bass kernel for nn_CrossAttention (tanh-scored, reversed-weight
attention), collective-free replicated-KV design, transposed epilogue.

Math (reference):
    q = x1 @ Wq.T + bq ; k = x2 @ Wk.T + bk ; v = x2 @ Wv.T + bv
    attn = softmax(tanh(q @ k.T) / sqrt(512), axis=-1)
    out  = ((1 - attn) / (N-1)) @ v

Kernel algebra:
    t_ij = tanh(q_i . k_j)                        (biases folded into q, k)
    e_ij = exp(scale * t_ij) ~= 1 + scale * t_ij  (|scale*t| <= 0.0442)
    r_i  = N + scale * R_i,  R_i = rowsum_j t_ij
    1/r_i ~= (1/N)(1 - scale*R_i/N)               (|scale*R/N| ~ 2e-6 rel err)
    out_{i,d} = C1_d + c3_d * R_i - k2 * av_{d,i}
      C1_d = cv_d/N + bv_d          (cv = colsum of v-without-bias, fp32 path)
      c3_d = scale*cv_d/(N^2 (N-1))
      k2   = scale/(N (N-1))
      av   = (t @ v)^T  accumulated TRANSPOSED in PSUM ([d partitions, i free])
    The c3_d*R_i correction is folded into the same PSUM accumulation as av
    via one rank-1 f32r matmul per [128,512] output tile (lhsT = c3row
    slice, rhs = R row); a single scalar-engine activation
    (out = -k2*psum + C1_d, per-partition bias) drains PSUM straight to the
    output tile.  v's column 0 is a ones-column, so av^T partition 0 of
    d-chunk 0 IS R_i -- no rowsum matmuls; the dropped true-v column
    contribution is ~2e-8 of out.  C1_col rides a tiny DRAM round-trip to
    reach the d-on-partitions layout the bias needs.

Sharding: rows of x_1 (queries) sharded across 8 cores; x_2/weights
replicated, each core projects full K/V locally; no collectives.

Output is produced transposed per (d-chunk, i-half) tile ([4,2,128,512])
and untransposed host-side in kernel().

DMA queue map (keeps next-iteration loads from stalling behind epilogue):
    sync:   wq, x1, bq loads, then x2 even blocks
    gpsimd: wk, wv, bk, bv, wv32 loads, x2 odd blocks, out DMAs (tail)
    vector: x2 casts cc<4, k drains, rs-row memset/DMA
    scalar: x2 casts cc>=4, v drains, q drains, tanh, cv/coef/epilogue drains
"""

import numpy as np
from contextlib import ExitStack

import concourse.bass as bass
import concourse.mybir as mybir
import concourse.tile as tile
from concourse import bacc
from concourse.bass_utils import run_bass_kernel_spmd

F32 = mybir.dt.float32
BF16 = mybir.dt.bfloat16
FP8 = mybir.dt.float8e4
F32R = mybir.dt.float32r

NCORES = 8
N = 8192             # total rows (keys/values)
CIN = 1024           # input feature dim
D = 512              # d_kq = d_v
P = 128              # partitions
S = N // NCORES      # query rows per core (1024)
NCC = CIN // P       # 8 feature chunks
NDC = D // P         # 4 d chunks
NJB = 16             # x2 streaming blocks
JB = N // NJB        # 512 j columns per block
NJC = N // P         # 64 j chunks
SCALE = float(1.0 / np.sqrt(np.float64(D)))
K2 = float(SCALE / (np.float64(N) * (N - 1)))          # av coefficient
CB = float(-1.0 / N)                                   # c3row coefficient
ACT_COPY = mybir.ActivationFunctionType.Copy
ACT_IDENT = mybir.ActivationFunctionType.Identity
ACT_TANH = mybir.ActivationFunctionType.Tanh
DR = mybir.MatmulPerfMode.DoubleRow


def emit_body(nc, tc, io, persist_tiles, pools, staged=False):
    """Emit one full kernel iteration (projections + attention + epilogue).

    The first four x2 block DMAs are hoisted ahead of the q-projection so
    they sit early in the trigger queues and the staggered-reset stage
    assignment, letting the next iteration's stream prefetch sooner.
    (Manual tc.stage_boundary() placements were tried twice and always
    regressed ~15us -- the per-stage barrier overhead exceeds the gain.)
    """
    x1t, x2t, wqt, wkt, wvt, wvt32, bqt, bkt, bvt, out = io
    (wpool, kvpool, tfull, loads8, loadsbf, epool, cspool, ps2,
     dpool) = pools

    # ---- weight / bias / x1 loads; q-side on sync (fires earliest) ----
    wq_sb = wpool.tile([P, NCC, D], FP8, tag="wq")
    wk_sb = wpool.tile([P, NCC, D], FP8, tag="wk")
    wv_sb = wpool.tile([P, NCC, D], FP8, tag="wv")
    wv32_sb = wpool.tile([P, NCC, D], F32R, tag="wv32")
    bq_sb = wpool.tile([P, NDC], F32, tag="bq")
    bk_sb = wpool.tile([P, NDC], F32, tag="bk")
    bvt_sb = wpool.tile([P, NDC], F32, tag="bvt")
    x1_sb = wpool.tile([P, NCC, S], FP8, tag="x1")
    nc.sync.dma_start(out=wq_sb, in_=wqt[:, :, :])
    nc.sync.dma_start(out=x1_sb, in_=x1t[:, :, :])
    nc.sync.dma_start(out=bq_sb, in_=bqt[:, :])
    nc.gpsimd.dma_start(out=wk_sb, in_=wkt[:, :, :])
    nc.gpsimd.dma_start(out=wv_sb, in_=wvt[:, :, :])
    nc.gpsimd.dma_start(out=bk_sb, in_=bkt[:, :])
    nc.gpsimd.dma_start(out=bvt_sb, in_=bvt[:, :])

    # first 4 x2 blocks load in stage 0 alongside the weights
    x2b_pre = []
    for jb in range(4):
        x2b = loadsbf.tile([P, NCC, JB], BF16, tag="x2b", name=f"x2b_p{jb}")
        if jb % 2 == 0:
            nc.sync.dma_start(out=x2b, in_=x2t[jb, :, :, :])
        else:
            nc.gpsimd.dma_start(out=x2b, in_=x2t[jb, :, :, :])
        x2b_pre.append(x2b)

    # ---- q projection: qt[d, i] fp8, bias folded ----
    qt = kvpool.tile([P, NDC, S], FP8, tag="qt")
    for di in range(NDC):
        pq = ps2.tile([P, 2, D], F32, tag="s2")
        for ih in range(2):
            for cp in range(NCC // 2):
                nc.tensor.matmul(
                    pq[:, ih, :],
                    lhsT=wq_sb[:, 2 * cp:2 * cp + 2, di * P:(di + 1) * P],
                    rhs=x1_sb[:, 2 * cp:2 * cp + 2, ih * D:(ih + 1) * D],
                    perf_mode=DR, start=(cp == 0), stop=(cp == NCC // 2 - 1))
        nc.scalar.activation(out=qt[:, di, :], in_=pq,
                             func=ACT_IDENT, bias=bq_sb[:, di:di + 1])

    # ---- streamed k/v projection over 16 j-blocks of 512 ----
    kt = kvpool.tile([P, NDC, N], FP8, tag="kt")        # kT[d, j]
    vv = kvpool.tile([P, NJC, D], FP8, tag="v")         # v[j, d] (no bias)
    # v column 0 is a ones-column: (t@v)^T then lands R_i (the rowsum of t)
    # at av^T partition 0 of d-chunk 0, where a plain (no partition shift)
    # activation copy can read it into the epilogue's rhs row 0.
    nc.vector.memset(vv[:, :, 0:1], 1.0)
    cs_part = cspool.tile([P, NCC, NJB], F32, tag="csp")
    for jb in range(NJB):
        if jb < 4:
            x2b = x2b_pre[jb]
        else:
            x2b = loadsbf.tile([P, NCC, JB], BF16, tag="x2b")
            # alternate queues so the 16 MB stream rides two DMA channels
            if jb % 2 == 0:
                nc.sync.dma_start(out=x2b, in_=x2t[jb, :, :, :])
            else:
                nc.gpsimd.dma_start(out=x2b, in_=x2t[jb, :, :, :])
        x2f = loads8.tile([P, NCC, JB], FP8, tag="x2f")
        # fused bf16->fp8 cast + f32-accumulated colsum via accum_out
        for cc in range(NCC):
            if cc < 4:
                nc.vector.tensor_scalar(
                    x2f[:, cc, :], x2b[:, cc, :], 1.0, 0.0,
                    op0=mybir.AluOpType.mult, op1=mybir.AluOpType.add,
                    accum_out=cs_part[:, cc, jb:jb + 1])
            else:
                nc.scalar.activation(
                    out=x2f[:, cc, :], in_=x2b[:, cc, :], func=ACT_COPY,
                    accum_out=cs_part[:, cc, jb:jb + 1])
        # kT block: [512 d, 512 j]
        for dp in range(NDC // 2):
            pk = ps2.tile([P, 2, D], F32, tag="s2")
            for dh in range(2):
                di = 2 * dp + dh
                for cp in range(NCC // 2):
                    nc.tensor.matmul(
                        pk[:, dh, :],
                        lhsT=wk_sb[:, 2 * cp:2 * cp + 2, di * P:(di + 1) * P],
                        rhs=x2f[:, 2 * cp:2 * cp + 2, :],
                        perf_mode=DR, start=(cp == 0),
                        stop=(cp == NCC // 2 - 1))
                nc.vector.tensor_scalar_add(
                    kt[:, di, jb * JB:(jb + 1) * JB], pk[:, dh, :],
                    bk_sb[:, di:di + 1])
        # v block: [512 j, 512 dv]
        for vp in range(JB // P // 2):
            pv = ps2.tile([P, 2, D], F32, tag="s2")
            for vh in range(2):
                jl = 2 * vp + vh
                for cp in range(NCC // 2):
                    nc.tensor.matmul(
                        pv[:, vh, :],
                        lhsT=x2f[:, 2 * cp:2 * cp + 2, jl * P:(jl + 1) * P],
                        rhs=wv_sb[:, 2 * cp:2 * cp + 2, :],
                        perf_mode=DR, start=(cp == 0),
                        stop=(cp == NCC // 2 - 1))
            jj = jb * (JB // P) + 2 * vp
            nc.scalar.activation(out=vv[:, jj:jj + 2, 1:D],
                                 in_=pv[:, :, 1:D], func=ACT_COPY)

    # wv32 is only needed now (cv); its load rides after the stream dispatches
    nc.gpsimd.dma_start(out=wv32_sb, in_=wvt32[:, :, :])

    # ---- colsum_v (fp32): cv1[1,512], then coef[2,512] via tiny PE matmuls
    cs = cspool.tile([P, NCC], F32R, tag="cs")
    with nc.allow_low_precision(reason="cv via TF32-rate matmul; ~1e-3 rel "
                                "err on the dominant term, tolerance 2e-2"):
        nc.vector.reduce_sum(out=cs, in_=cs_part, axis=mybir.AxisListType.X)
    ps_cv = ps2.tile([P, 2, D], F32, tag="s2")
    for ci in range(NCC):
        nc.tensor.matmul(ps_cv[0:1, 0, :], lhsT=cs[:, ci:ci + 1],
                         rhs=wv32_sb[:, ci, :],
                         start=(ci == 0), stop=(ci == NCC - 1))
    cv1 = cspool.tile([1, D], F32, tag="cv1")
    nc.scalar.activation(out=cv1, in_=ps_cv[0:1, 0, :], func=ACT_COPY)
    # c3row = -cv/N (rank-1 coefficient row, partition 0)
    c3row = cspool.tile([1, D], F32R, tag="c3row")
    nc.vector.tensor_scalar_mul(c3row, cv1, CB)
    # C1_col[128, NDC] = cv^T/N + bv^T; the [1,512]->[128,4] transpose rides
    # a tiny DRAM round-trip (DMA reshapes across partitions freely)
    cv_dram = dpool.tile([1, D], F32, name="cvd")
    nc.scalar.dma_start(out=cv_dram, in_=cv1)
    cvt_col = cspool.tile([P, NDC], F32, tag="cvtc")
    for dc in range(NDC):
        nc.scalar.dma_start(out=cvt_col[:, dc:dc + 1],
                            in_=cv_dram[0:1, dc * P:(dc + 1) * P])
    c1col = cspool.tile([P, NDC], F32, tag="c1col")
    nc.vector.tensor_scalar_mul(c1col, cvt_col, float(1.0 / N))
    nc.vector.tensor_add(c1col, c1col, bvt_sb)

    # ---- main attention loop, one i-half (512 rows) at a time ----
    for ih in range(2):
        ps_av = [ps2.tile([P, 2, D], F32, tag="s2", name=f"av{ap}_{ih}")
                 for ap in range(2)]
        t2f = tfull.tile([P, NJC, D], FP8, tag="t2f")
        for jp in range(NJC // 2):
            ps_s = ps2.tile([P, 2, D], F32, tag="s2")
            for sh in range(2):
                jc = 2 * jp + sh
                for qp in range(2):
                    nc.tensor.matmul(
                        ps_s[:, sh, :],
                        lhsT=kt[:, 2 * qp:2 * qp + 2, jc * P:(jc + 1) * P],
                        rhs=qt[:, 2 * qp:2 * qp + 2, ih * D:(ih + 1) * D],
                        perf_mode=DR, start=(qp == 0), stop=(qp == 1))
            nc.scalar.activation(out=t2f[:, 2 * jp:2 * jp + 2, :], in_=ps_s,
                                 func=ACT_TANH)
            # av^T accumulation: [d partitions, i free]
            for dc in range(NDC):
                nc.tensor.matmul(
                    ps_av[dc // 2][:, dc % 2, :],
                    lhsT=vv[:, 2 * jp:2 * jp + 2, dc * P:(dc + 1) * P],
                    rhs=t2f[:, 2 * jp:2 * jp + 2, :],
                    perf_mode=DR, start=(jp == 0), stop=False)

        # ---- epilogue: R_i sits at av^T partition 0 of d-chunk 0 ----
        rs_row = epool.tile([1, D], F32R, tag="rs")
        nc.scalar.activation(out=rs_row, in_=ps_av[0][0:1, 0, :],
                             func=ACT_COPY)
        for dc in range(NDC):
            pa = ps_av[dc // 2][:, dc % 2, :]
            # fold the rank-1 R-term into the accumulation, then drain with
            # out = -K2 * psum + C1_d  (per-partition bias) on the scalar eng
            nc.tensor.matmul(pa, lhsT=c3row[0:1, dc * P:(dc + 1) * P],
                             rhs=rs_row, start=False, stop=True)
            o1 = epool.tile([P, D], F32, tag="o1")
            # drains split across engines so all four PSUM banks free fast
            # (the next iteration's q-projection reuses them)
            if dc < 2:
                nc.vector.tensor_scalar(
                    o1, pa, -K2, c1col[:, dc:dc + 1],
                    op0=mybir.AluOpType.mult, op1=mybir.AluOpType.add)
            else:
                nc.scalar.activation(out=o1, in_=pa, func=ACT_IDENT,
                                     scale=-K2, bias=c1col[:, dc:dc + 1])
            nc.gpsimd.dma_start(out=out[dc, ih, :, :], in_=o1)


def build_kernel(repeat: int = 1):
    nc = bacc.Bacc(num_devices=NCORES)

    x1t = nc.declare_dram_parameter("x1t", [P, NCC, S], FP8, isOutput=False)
    x2t = nc.declare_dram_parameter("x2t", [NJB, P, NCC, JB], BF16,
                                    isOutput=False)
    wqt = nc.declare_dram_parameter("wqt", [P, NCC, D], FP8, isOutput=False)
    wkt = nc.declare_dram_parameter("wkt", [P, NCC, D], FP8, isOutput=False)
    wvt = nc.declare_dram_parameter("wvt", [P, NCC, D], FP8, isOutput=False)
    wvt32 = nc.declare_dram_parameter("wvt32", [P, NCC, D], F32, isOutput=False)
    bqt = nc.declare_dram_parameter("bqt", [P, NDC], F32, isOutput=False)
    bkt = nc.declare_dram_parameter("bkt", [P, NDC], F32, isOutput=False)
    bvt = nc.declare_dram_parameter("bvt", [P, NDC], F32, isOutput=False)
    # transposed output tiles: [d-chunk, i-half, 128 d, 512 i]
    out = nc.declare_dram_parameter("out", [NDC, 2, P, D], F32, isOutput=True)
    io = (x1t, x2t, wqt, wkt, wvt, wvt32, bqt, bkt, bvt, out)

    with tile.TileContext(nc) as tc, ExitStack() as ctx:
        persist_tiles = ()

        wpool = ctx.enter_context(tc.tile_pool(name="weights", bufs=1))
        kvpool = ctx.enter_context(tc.tile_pool(name="kv", bufs=1))
        tfull = ctx.enter_context(tc.tile_pool(name="tfull", bufs=1))
        loads8 = ctx.enter_context(tc.tile_pool(name="loads8", bufs=4))
        loadsbf = ctx.enter_context(tc.tile_pool(name="loadsbf", bufs=4))
        epool = ctx.enter_context(tc.tile_pool(name="epool", bufs=2))
        cspool = ctx.enter_context(tc.tile_pool(name="cspool", bufs=1))
        ps2 = ctx.enter_context(tc.tile_pool(name="ps2", bufs=4, space="PSUM"))
        dpool = ctx.enter_context(tc.tile_pool(name="dram", bufs=2,
                                               space="DRAM"))
        pools = (wpool, kvpool, tfull, loads8, loadsbf, epool, cspool, ps2,
                 dpool)

        if repeat == 1:
            emit_body(nc, tc, io, persist_tiles, pools)
        else:
            with tc.For_i(0, repeat, 1,
                          staggered_reset=True,
                          hint_engines=(mybir.EngineType.PE,
                                        mybir.EngineType.Activation,
                                        mybir.EngineType.DVE,
                                        mybir.EngineType.SP,
                                        mybir.EngineType.Pool)):
                emit_body(nc, tc, io, persist_tiles, pools, staged=True)

    if not nc.is_finalized():
        nc.finalize()
    return nc


_NC_CACHE = {}


def _get_nc(repeat: int = 1):
    if repeat not in _NC_CACHE:
        _NC_CACHE[repeat] = build_kernel(repeat)
    return _NC_CACHE[repeat]


def make_in_maps(x_1, x_2, Wq, bq, Wk, bk, Wv, bv):
    f8 = mybir.dt.np(FP8)
    bf = mybir.dt.np(BF16)

    def chunked_t(a, dtype):
        # [rows, cin] -> transposed, feature-chunked [128, cin//128, rows]
        a = np.asarray(a, np.float32)
        cin, rows = a.shape[1], a.shape[0]
        return np.ascontiguousarray(
            a.T.reshape(cin // P, P, rows).transpose(1, 0, 2)).astype(dtype)

    def blocked(a):
        # [128, 8, N] -> j-blocked [16, 128, 8, 512] (contiguous per block)
        return np.ascontiguousarray(
            a.reshape(P, NCC, NJB, JB).transpose(2, 0, 1, 3))

    x1t = chunked_t(x_1, f8)                      # [128, 8, 8192]
    shared = {
        "x2t": blocked(chunked_t(x_2, bf)),
        "wqt": chunked_t(np.asarray(Wq), f8),     # [128, 8, 512]
        "wkt": chunked_t(np.asarray(Wk), f8),
        "wvt": chunked_t(np.asarray(Wv), f8),
        "wvt32": chunked_t(np.asarray(Wv), np.float32),
        "bqt": np.ascontiguousarray(
            np.asarray(bq, np.float32).reshape(NDC, P).T),
        "bkt": np.ascontiguousarray(
            np.asarray(bk, np.float32).reshape(NDC, P).T),
        "bvt": np.ascontiguousarray(
            np.asarray(bv, np.float32).reshape(NDC, P).T),
    }
    return [
        {"x1t": np.ascontiguousarray(x1t[:, :, c * S:(c + 1) * S]), **shared}
        for c in range(NCORES)
    ]


def untranspose_out(o):
    # [NDC, 2, 128 d, 512 i] -> [1024 i, 512 d]
    return np.ascontiguousarray(
        o.transpose(1, 3, 0, 2).reshape(S, D))


def kernel(x_1, x_2, Wq, bq, Wk, bk, Wv, bv):
    nc = _get_nc(1)
    in_maps = make_in_maps(x_1, x_2, Wq, bq, Wk, bk, Wv, bv)
    res = run_bass_kernel_spmd(nc, in_maps, core_ids=list(range(NCORES)))
    return np.concatenate(
        [untranspose_out(res.results[c]["out"]) for c in range(NCORES)],
        axis=0)
